# revision 1
# baseline (speedup 1.0000x reference)
"""Trainium2 Bass kernel for nn_Erode (5x5 all-ones SE, zero padding).

For an all-ones 5x5 structuring element, kornia-style Erode reduces to a
5x5 sliding-window MIN over the zero-padded image.  The min is separable:
a 5-tap vertical pass then a 5-tap horizontal pass, each done with 3
tensor_tensor(min) ops (pairwise / skip-2 / final tap).

Distribution: pure data parallel.  B*C = 24 images of 512x512 are split
3-per-core across 8 NeuronCores.  Inside a core, the 3 images' 1536 rows
are striped over the 128 SBUF partitions: partition p of image-group i
owns K=13 output rows (40 partitions per image, 8 idle).  Each partition
loads its 13 rows plus a 2-row halo on both sides (17 row-slots) from a
host-side zero-padded copy of the input, so both min passes are pure
free-dimension sliding ops (engine ops on TRN2 cannot read partition-
shifted operands).  Work is chunked along W for DMA/compute overlap.
"""

import numpy as np

# ---- fixed problem geometry (hardcoded per harness contract) ----
B, C, H, W = 8, 3, 512, 512
N_CORES = 8
IMGS_PER_CORE = (B * C) // N_CORES  # 3
K = 13                   # output rows per partition
SLOTS = K + 4            # row-slots incl. 2+2 halo
P_PER_IMG = 40           # ceil(512/13); last partition owns 5 rows
PAD_H = 2 + H + 10       # 524: 2 top pad + data + pad tail (>= slots overrun)
PAD_W = 2 + W + 2        # 516
CW = 128                 # W-chunk of output columns
NCH = W // CW            # 4

_cached = {}


def _build_program():
    import concourse.mybir as mybir
    from concourse import bass, bacc
    from concourse.tile import TileContext

    f32 = mybir.dt.float32
    MIN = mybir.AluOpType.min

    nc = bacc.Bacc("TRN2", target_bir_lowering=False, debug=False,
                   num_devices=N_CORES)
    xp = nc.dram_tensor("xp", [IMGS_PER_CORE * PAD_H * PAD_W], f32,
                        kind="ExternalInput")
    y = nc.dram_tensor("y", [IMGS_PER_CORE * H * W], f32,
                       kind="ExternalOutput")

    with TileContext(nc) as tc:
        with tc.tile_pool(name="work", bufs=3) as pool:
            for ch in range(NCH):
                c0 = CW * ch
                lw = CW + 4  # loaded cols incl. 2+2 halo
                X = pool.tile([128, SLOTS, lw], f32, tag="X")
                for i in range(IMGS_PER_CORE):
                    src = bass.AP(
                        tensor=xp,
                        offset=i * PAD_H * PAD_W + c0,
                        ap=[[K * PAD_W, P_PER_IMG], [PAD_W, SLOTS], [1, lw]],
                    )
                    nc.sync.dma_start(
                        out=X[P_PER_IMG * i:P_PER_IMG * (i + 1)], in_=src)

                # vertical 5-tap min along row-slots
                P = pool.tile([128, SLOTS - 1, lw], f32, tag="P")
                nc.vector.tensor_tensor(out=P, in0=X[:, 0:16], in1=X[:, 1:17],
                                        op=MIN)
                Q = pool.tile([128, SLOTS - 3, lw], f32, tag="Q")
                nc.vector.tensor_tensor(out=Q, in0=P[:, 0:14], in1=P[:, 2:16],
                                        op=MIN)
                V = pool.tile([128, K, lw], f32, tag="V")
                nc.vector.tensor_tensor(out=V, in0=Q[:, 0:13], in1=X[:, 4:17],
                                        op=MIN)

                # horizontal 5-tap min along cols
                P2 = pool.tile([128, K, lw - 1], f32, tag="P")
                nc.vector.tensor_tensor(out=P2, in0=V[:, :, 0:lw - 1],
                                        in1=V[:, :, 1:lw], op=MIN)
                Q2 = pool.tile([128, K, lw - 3], f32, tag="Q")
                nc.vector.tensor_tensor(out=Q2, in0=P2[:, :, 0:lw - 3],
                                        in1=P2[:, :, 2:lw - 1], op=MIN)
                Hm = pool.tile([128, K, CW], f32, tag="V2")
                nc.vector.tensor_tensor(out=Hm, in0=Q2[:, :, 0:CW],
                                        in1=V[:, :, 4:lw], op=MIN)

                for i in range(IMGS_PER_CORE):
                    pb = P_PER_IMG * i
                    dst_a = bass.AP(
                        tensor=y,
                        offset=(H * i) * W + c0,
                        ap=[[K * W, P_PER_IMG - 1], [W, K], [1, CW]],
                    )
                    nc.sync.dma_start(out=dst_a, in_=Hm[pb:pb + P_PER_IMG - 1])
                    dst_b = bass.AP(
                        tensor=y,
                        offset=(H * i + K * (P_PER_IMG - 1)) * W + c0,
                        ap=[[K * W, 1], [W, H - K * (P_PER_IMG - 1)], [1, CW]],
                    )
                    nc.sync.dma_start(
                        out=dst_b,
                        in_=Hm[pb + P_PER_IMG - 1:pb + P_PER_IMG,
                               0:H - K * (P_PER_IMG - 1)])
    nc.compile()
    return nc


def _get_program():
    if "nc" not in _cached:
        _cached["nc"] = _build_program()
    return _cached["nc"]


def _pad_core_input(x3: np.ndarray) -> np.ndarray:
    """[3,512,512] -> zero-padded flat [3*524*516]."""
    xp = np.zeros((IMGS_PER_CORE, PAD_H, PAD_W), np.float32)
    xp[:, 2:2 + H, 2:2 + W] = x3
    return xp.reshape(-1)


def _run_on_hw(x24: np.ndarray, trace: bool = False):
    from concourse.bass_utils import run_bass_kernel_spmd
    nc = _get_program()
    in_maps = [
        {"xp": _pad_core_input(x24[IMGS_PER_CORE * k:IMGS_PER_CORE * (k + 1)])}
        for k in range(N_CORES)
    ]
    res = run_bass_kernel_spmd(nc, in_maps, list(range(N_CORES)), trace=trace)
    out = np.stack([
        res.results[k]["y"].reshape(IMGS_PER_CORE, H, W)
        for k in range(N_CORES)
    ])
    return out.reshape(B, C, H, W), res


def _erode_reference_np(x: np.ndarray, se: np.ndarray) -> np.ndarray:
    """Generic fallback faithful to the kornia-style formula (numpy)."""
    kh, kw = se.shape
    ph, pw = kh // 2, kw // 2
    xpad = np.pad(x, ((0, 0), (0, 0), (ph, ph), (pw, pw)))
    out = None
    for r in range(kh):
        for c in range(kw):
            shifted = xpad[:, :, r:r + x.shape[2], c:c + x.shape[3]]
            bias = se[r, c] - 1.0
            val = shifted - bias if bias >= 0.0 else np.full_like(shifted, -bias)
            out = val if out is None else np.minimum(out, val)
    return out.astype(x.dtype)


def kernel(x, se):
    x = np.asarray(x, dtype=np.float32)
    se = np.asarray(se, dtype=np.float32)
    if se.shape != (5, 5) or not np.all(se == 1.0) or x.shape != (B, C, H, W):
        return _erode_reference_np(x, se)
    x24 = np.ascontiguousarray(x.reshape(B * C, H, W))
    out, _ = _run_on_hw(x24, trace=False)
    return out


# revision 5
# speedup vs baseline: 1.1149x; 1.1149x over previous
"""Trainium2 Bass kernel for nn_Erode (5x5 all-ones SE, zero padding).

For an all-ones 5x5 structuring element, kornia-style Erode reduces to a
5x5 sliding-window MIN over the zero-padded image.  The min is separable:
a 5-tap vertical pass then a 5-tap horizontal pass, each done with 3
tensor_tensor(min) ops (pairwise / skip-2 / final tap).

Distribution: pure data parallel.  B*C = 24 images of 512x512 are split
3-per-core across 8 NeuronCores.  Inside a core, the 3 images' 1536 rows
are striped over the 128 SBUF partitions: partition p of image-group i
owns K=13 output rows (40 partitions per image, 8 idle).  Each partition
loads its 13 rows plus a 2-row halo on both sides (17 row-slots) from a
host-side zero-padded copy of the input, so both min passes are pure
free-dimension sliding ops (engine ops on TRN2 cannot read partition-
shifted operands).  Work is chunked along W for DMA/compute overlap.
"""

import numpy as np

# ---- fixed problem geometry (hardcoded per harness contract) ----
B, C, H, W = 8, 3, 512, 512
N_CORES = 8
IMGS_PER_CORE = (B * C) // N_CORES  # 3
K = 13                   # output rows per partition
SLOTS = K + 4            # row-slots incl. 2+2 halo
P_PER_IMG = 40           # ceil(512/13); last partition owns 5 rows
PAD_H = 2 + H + 10       # 524: 2 top pad + data + pad tail (>= slots overrun)
PAD_W = 2 + W + 2        # 516
CW = 256                 # W-chunk of output columns
NCH = W // CW            # 2

_cached = {}


def _build_program():
    import concourse.mybir as mybir
    from concourse import bass, bacc
    from concourse.tile import TileContext

    f32 = mybir.dt.float32
    MIN = mybir.AluOpType.min

    nc = bacc.Bacc("TRN2", target_bir_lowering=False, debug=False,
                   num_devices=N_CORES)
    xp = nc.dram_tensor("xp", [IMGS_PER_CORE * PAD_H * PAD_W], f32,
                        kind="ExternalInput")
    y = nc.dram_tensor("y", [IMGS_PER_CORE * H * W], f32,
                       kind="ExternalOutput")

    # HWDGE-capable issuers; spreading DMAs over them uses parallel queues
    dma_engines = [nc.sync, nc.scalar, nc.sync]

    with TileContext(nc) as tc:
        with tc.tile_pool(name="work", bufs=2) as pool:
            for ch in range(NCH):
                c0 = CW * ch
                lw = CW + 4  # loaded cols incl. 2+2 halo
                X = pool.tile([128, SLOTS, lw], f32, tag="X")
                for i in range(IMGS_PER_CORE):
                    src = bass.AP(
                        tensor=xp,
                        offset=i * PAD_H * PAD_W + c0,
                        ap=[[K * PAD_W, P_PER_IMG], [PAD_W, SLOTS], [1, lw]],
                    )
                    dma_engines[i].dma_start(
                        out=X[P_PER_IMG * i:P_PER_IMG * (i + 1)], in_=src)

                # vertical 5-tap min along row-slots
                P = pool.tile([128, SLOTS - 1, lw], f32, tag="P")
                nc.vector.tensor_tensor(out=P, in0=X[:, 0:16], in1=X[:, 1:17],
                                        op=MIN)
                Q = pool.tile([128, SLOTS - 3, lw], f32, tag="Q")
                nc.vector.tensor_tensor(out=Q, in0=P[:, 0:14], in1=P[:, 2:16],
                                        op=MIN)
                V = pool.tile([128, K, lw], f32, tag="V")
                nc.vector.tensor_tensor(out=V, in0=Q[:, 0:13], in1=X[:, 4:17],
                                        op=MIN)

                # horizontal 5-tap min along cols
                P2 = pool.tile([128, K, lw - 1], f32, tag="P")
                nc.vector.tensor_tensor(out=P2, in0=V[:, :, 0:lw - 1],
                                        in1=V[:, :, 1:lw], op=MIN)
                Q2 = pool.tile([128, K, lw - 3], f32, tag="Q")
                nc.vector.tensor_tensor(out=Q2, in0=P2[:, :, 0:lw - 3],
                                        in1=P2[:, :, 2:lw - 1], op=MIN)
                Hm = pool.tile([128, K, CW], f32, tag="V2")
                nc.vector.tensor_tensor(out=Hm, in0=Q2[:, :, 0:CW],
                                        in1=V[:, :, 4:lw], op=MIN)

                for i in range(IMGS_PER_CORE):
                    pb = P_PER_IMG * i
                    eng = dma_engines[(i + ch) % len(dma_engines)]
                    dst_a = bass.AP(
                        tensor=y,
                        offset=(H * i) * W + c0,
                        ap=[[K * W, P_PER_IMG - 1], [W, K], [1, CW]],
                    )
                    eng.dma_start(out=dst_a, in_=Hm[pb:pb + P_PER_IMG - 1])
                    dst_b = bass.AP(
                        tensor=y,
                        offset=(H * i + K * (P_PER_IMG - 1)) * W + c0,
                        ap=[[K * W, 1], [W, H - K * (P_PER_IMG - 1)], [1, CW]],
                    )
                    eng.dma_start(
                        out=dst_b,
                        in_=Hm[pb + P_PER_IMG - 1:pb + P_PER_IMG,
                               0:H - K * (P_PER_IMG - 1)])
    nc.compile()
    return nc


def _get_program():
    if "nc" not in _cached:
        _cached["nc"] = _build_program()
    return _cached["nc"]


def _pad_core_input(x3: np.ndarray) -> np.ndarray:
    """[3,512,512] -> zero-padded flat [3*524*516]."""
    xp = np.zeros((IMGS_PER_CORE, PAD_H, PAD_W), np.float32)
    xp[:, 2:2 + H, 2:2 + W] = x3
    return xp.reshape(-1)


def _run_on_hw(x24: np.ndarray, trace: bool = False):
    from concourse.bass_utils import run_bass_kernel_spmd
    nc = _get_program()
    in_maps = [
        {"xp": _pad_core_input(x24[IMGS_PER_CORE * k:IMGS_PER_CORE * (k + 1)])}
        for k in range(N_CORES)
    ]
    res = run_bass_kernel_spmd(nc, in_maps, list(range(N_CORES)), trace=trace)
    out = np.stack([
        res.results[k]["y"].reshape(IMGS_PER_CORE, H, W)
        for k in range(N_CORES)
    ])
    return out.reshape(B, C, H, W), res


def _erode_reference_np(x: np.ndarray, se: np.ndarray) -> np.ndarray:
    """Generic fallback faithful to the kornia-style formula (numpy)."""
    kh, kw = se.shape
    ph, pw = kh // 2, kw // 2
    xpad = np.pad(x, ((0, 0), (0, 0), (ph, ph), (pw, pw)))
    out = None
    for r in range(kh):
        for c in range(kw):
            shifted = xpad[:, :, r:r + x.shape[2], c:c + x.shape[3]]
            bias = se[r, c] - 1.0
            val = shifted - bias if bias >= 0.0 else np.full_like(shifted, -bias)
            out = val if out is None else np.minimum(out, val)
    return out.astype(x.dtype)


def kernel(x, se):
    x = np.asarray(x, dtype=np.float32)
    se = np.asarray(se, dtype=np.float32)
    if se.shape != (5, 5) or not np.all(se == 1.0) or x.shape != (B, C, H, W):
        return _erode_reference_np(x, se)
    x24 = np.ascontiguousarray(x.reshape(B * C, H, W))
    out, _ = _run_on_hw(x24, trace=False)
    return out
